# revision 39
# baseline (speedup 1.0000x reference)
"""Trainium2 Bass kernel for a dense transformer encoder layer.

Model dims: B=4, S=2048, D=512, H=8 heads, E=64 head dim, F=2048 ffn dim.

Sharding: 8 cores, core c -> (batch b = c//2, sequence half = c%2).
Each core receives its batch's full 2048 tokens (reordered so the core's
1024 query rows come first) and computes the full layer for its 1024
query tokens; K/V are computed for all 2048 tokens on-core, so no
cross-core communication is needed (softmax over keys is permutation
invariant, so the sequence reorder is harmless).

Layer math on one core (q = 1024 query tokens, k = 2048 kv tokens):
  norm1 (layernorm, Bessel std) -> x_norm^T [D, k] bf16 (PE transposes)
  Q^T/K^T = W_{q,k}^T x_norm^T (+bias, per-partition)   [HE, q|k]
  V      = x_norm W_v (+bias) stored [k, H*(E+1)] with a ones column per
           head so the attention GEMM also produces the softmax row sums
  scores^T = K_h Q_h^T (K=64 contraction), exp on ScalarE (scale=1/8)
  att^T[e,q](+sums row) = V_aug^T exp^T accumulated over k tiles
  normalize: recip(sums) -> K=1 matmul broadcast -> multiply
  att_out = att_norm^T^T Wp; x1 = att_out + x + bp; norm2; FFN with
  exact Gelu on both FFN outputs; y = gelu2 + x1.

gamma/beta of both norms are folded into the adjacent GEMM weights on the
host.  All GEMMs run in bf16 with fp32 PSUM accumulation.
"""

import numpy as np
import ml_dtypes

B, S, D, H, E, F = 4, 2048, 512, 8, 64, 2048
P = 128
SQ = S // 2          # query tokens per core
NQT = SQ // P        # 8 query 128-tiles
NKT = S // P         # 16 kv 128-tiles
C = D // P           # 4 chunks of the model dim
FC = F // P          # 16 chunks of the ffn dim
EA = E + 1           # head dim + ones column
SCALE = 1.0 / np.sqrt(E)
BESSEL = D / (D - 1.0)  # ddof=1 correction on variance

BF16 = ml_dtypes.bfloat16

# exp(s/8) = p(s)^32 with p a deg-3 fit of exp(s/256) over |s/256|<=0.23;
# runs on the Vector engine to offload softmax exp from ScalarE
EC1, EC2, EC3 = 3.90639966e-03, 7.65718235e-06, 9.89457506e-09

_CACHE = {}

# tuning knobs (swept via t_sweep.py)
CFG = {
    "ps_big_bufs": 2,    # scores/proj/ffn psum slots (2 banks each)
    "ps_att_bufs": 2,    # att accumulator slots (2 banks each)
    "v_pool": "att",     # which pool V-projection psums come from
    "tr_pool": "att",    # which pool transpose psums come from
    "dve_exp_mod": 0,    # kt % mod == mod-1 goes to DVE; 0 = ACT only
    "swpipe": True,      # delay att GEMMs one kt behind exp
    "scs_alt": True,     # alternate score tiles between psum pools
    "norm_eng": "dve",   # engine for the softmax-normalize copy/mult
    "order": "0011",
    "px_bufs": 5,
    "pxn_bufs": 3,
    "ptmp_bufs": 2,
    "pexp_bufs": 3,
}


def _register_dve_exp():
    import numpy as _np
    from concourse import dve_ops as DO
    from concourse.dve_spec import Spec, Src0, C0, C1, C2, One, sq, lower
    from concourse.dve_ops import has_src1
    from concourse.dve_uop import DveOpSpec

    if "EXP32_POLY_ANT" in DO._SUB_OPCODE_FOR_NAME:
        by = {op.name: op for op in DO.OPS}
        return by["EXP32_POLY_ANT"], by["EXP32_SQ_ANT"]

    s = Src0
    specs = [
        ("EXP32_POLY_ANT", Spec(
            body=((s * C2 + C1) * s + C0) * s + One,
            reference=lambda in0, in1, s0, s1, imm2: (
                (in0 * imm2 + s1) * in0 + s0) * in0 + 1.0)),
        ("EXP32_SQ_ANT", Spec(
            body=sq(sq(sq(sq(sq(s))))),
            reference=lambda in0, in1, s0, s1, imm2: (
                in0.astype(_np.float64) ** 32))),
    ]
    ops = []
    for name, spec in specs:
        op = DO.DveOp(name, spec, subdim=False, uops_sha={})
        DO.OPS.append(op)
        DO._SUB_OPCODE_FOR_NAME[name] = DO._CUSTOM_DVE_ROW_BASE + len(DO.OPS) - 1
        DO.CUSTOM_DVE_SPECS[name] = spec
        so = DveOpSpec(name=name, opcode=DO.get_dve_sub_opcode(name),
                       uops=lower(spec, ver="v3"), rd1_en=has_src1(spec))
        op.uops_sha["v3"] = so.sha("v3")
        ops.append(op)
    assert max(DO._SUB_OPCODE_FOR_NAME.values()) < 0x20
    return ops[0], ops[1]


def _build_program():
    """Build (and cache) the SPMD Bass program. Returns (nc, names)."""
    from contextlib import ExitStack

    import concourse.bass as bass
    import concourse.mybir as mybir
    import concourse.tile as tile
    from concourse import bacc

    f32 = mybir.dt.float32
    bf16 = mybir.dt.bfloat16
    AF = mybir.ActivationFunctionType
    OP = mybir.AluOpType

    xp_op, xs_op = _register_dve_exp()

    nc = bacc.Bacc(None, target_bir_lowering=False)

    # ---- DRAM I/O ----------------------------------------------------
    x_all = nc.dram_tensor("x_all", [P, NKT, D], f32, kind="ExternalInput")
    xqbp = nc.dram_tensor("xqbp", [P, NQT, D], f32, kind="ExternalInput")
    wq_d = nc.dram_tensor("wq", [P, C, H * E], bf16, kind="ExternalInput")
    wk_d = nc.dram_tensor("wk", [P, C, H * E], bf16, kind="ExternalInput")
    wv_d = nc.dram_tensor("wv", [P, C, H * E], bf16, kind="ExternalInput")
    wp_d = nc.dram_tensor("wp", [P, C, D], bf16, kind="ExternalInput")
    w1_d = nc.dram_tensor("w1", [P, C, F], bf16, kind="ExternalInput")
    w2_d = nc.dram_tensor("w2", [P, FC, D], bf16, kind="ExternalInput")
    bq_d = nc.dram_tensor("bq_c", [P, C], f32, kind="ExternalInput")
    bk_d = nc.dram_tensor("bk_c", [P, C], f32, kind="ExternalInput")
    bv_d = nc.dram_tensor("bv_b", [P, H * E], f32, kind="ExternalInput")
    b1_d = nc.dram_tensor("b1_c", [P, FC], f32, kind="ExternalInput")
    b2_d = nc.dram_tensor("b2_b", [P, D], f32, kind="ExternalInput")
    id_d = nc.dram_tensor("ident", [P, P], bf16, kind="ExternalInput")
    f32r = mybir.dt.float32r
    on_d = nc.dram_tensor("ones64", [1, E], f32r, kind="ExternalInput")
    y_out = nc.dram_tensor("y_out", [P, NQT, D], f32, kind="ExternalOutput")

    with tile.TileContext(nc) as tc, ExitStack() as ctx:
        pers = ctx.enter_context(tc.tile_pool(name="pers", bufs=1))
        px = ctx.enter_context(tc.tile_pool(name="px", bufs=CFG["px_bufs"]))
        pxn = ctx.enter_context(tc.tile_pool(name="pxn", bufs=CFG["pxn_bufs"]))
        pexp = ctx.enter_context(tc.tile_pool(name="pexp", bufs=CFG["pexp_bufs"]))
        ptmp = ctx.enter_context(tc.tile_pool(name="ptmp", bufs=CFG["ptmp_bufs"]))
        pst = ctx.enter_context(tc.tile_pool(name="pst", bufs=8))
        prr = ctx.enter_context(tc.tile_pool(name="prr", bufs=1))
        ps_big = ctx.enter_context(tc.tile_pool(name="ps_big", bufs=CFG["ps_big_bufs"], space="PSUM"))
        ps_att = ctx.enter_context(tc.tile_pool(name="ps_att", bufs=CFG["ps_att_bufs"], space="PSUM"))

        # ---- persistent SBUF tensors --------------------------------
        def pt(shape, dt, tag):
            return pers.tile(shape, dt, tag=tag, name=tag)

        w_q = pt([P, C, H * E], bf16, "w_q")
        w_k = pt([P, C, H * E], bf16, "w_k")
        w_v = pt([P, C, H * E], bf16, "w_v")
        w_p = pt([P, C, D], bf16, "w_p")
        w_1 = pt([P, C, F], bf16, "w_1")
        w_2 = pt([P, FC, D], bf16, "w_2")
        bq_c = pt([P, C], f32, "bq_c")
        bk_c = pt([P, C], f32, "bk_c")
        bv_b = pt([P, H * E], f32, "bv_b")
        b1_c = pt([P, FC], f32, "b1_c")
        b2_b = pt([P, D], f32, "b2_b")
        ident = pt([P, P], bf16, "ident")
        ones64 = pt([1, E], f32r, "ones64")
        xnT = pt([P, C, S], bf16, "xnT")
        qT = pt([P, C, SQ], bf16, "qT")
        kT = pt([P, C, S], bf16, "kT")
        v_sb = pt([P, NKT, H * EA], bf16, "v_sb")
        attnT = pt([P, C, SQ], bf16, "attnT")
        x1_sb = pt([P, NQT, D], f32, "x1_sb")
        x1nT = pt([P, C, SQ], bf16, "x1nT")
        hT = pt([P, FC, SQ], bf16, "hT")

        for dst, src in [
            (ident, id_d), (w_q, wq_d), (w_k, wk_d), (w_v, wv_d),
            (bq_c, bq_d), (bk_c, bk_d), (bv_b, bv_d), (ones64, on_d),
            (w_p, wp_d), (b1_c, b1_d), (b2_b, b2_d),
            (w_1, w1_d), (w_2, w2_d),
        ]:
            nc.sync.dma_start(dst[:], src[:])

        # ---- helper: layernorm stats -> (mean, rstd) ----------------
        def norm_stats(xt):
            st6 = pst.tile([P, 6], f32, tag="st6", name="st6")
            nc.vector.bn_stats(st6[:], xt)
            mv = pst.tile([P, 2], f32, tag="mv", name="mv")
            nc.vector.bn_aggr(mv[:], st6[:])
            std = pst.tile([P, 1], f32, tag="std", name="std")
            nc.scalar.activation(std[:], mv[:, 1:2], AF.Sqrt, scale=BESSEL)
            rstd = pst.tile([P, 1], f32, tag="rstd", name="rstd")
            nc.vector.reciprocal(rstd[:], std[:])
            return mv, rstd

        # transpose a [P, D] bf16 tile into dstT[:, :, tcol*P : +P]
        def transpose_into(dstT, xn, tcol):
            if CFG["tr_pool"] == "att":
                ps = ps_att.tile([P, 512], bf16, tag="att", name="tr")
            else:
                ps = ps_big.tile([P, 512], bf16, tag="mm", name="tr")
            for c in range(C):
                nc.tensor.transpose(
                    ps[:, c * P:(c + 1) * P], xn[:, c * P:(c + 1) * P], ident[:]
                )
            nc.scalar.copy(
                dstT[:, :, tcol * P:(tcol + 1) * P],
                ps[:].rearrange("p (c j) -> p c j", c=C),
            )

        # ---- phase A: norm1 + transpose ------------------------------
        for t in range(NKT):
            xt = px.tile([P, D], f32, tag="x", name="x")
            nc.gpsimd.dma_start(xt[:], x_all[:, t, :])
            mv, rstd = norm_stats(xt[:])
            xn = pxn.tile([P, D], bf16, tag="xn", name="xn")
            nc.gpsimd.tensor_scalar(
                xn[:], xt[:], mv[:, 0:1], rstd[:], OP.subtract, OP.mult
            )
            transpose_into(xnT, xn[:], t)

        # ---- phase B: QKV projections -------------------------------
        # Q^T / K^T: [HE, tokens] = sum_c W[:,c,:].T @ xnT[:,c,:]
        def proj_qk(w, dstT, bias_c, co, n0, ntiles):
            # [128, 1024] psum = two 512-wide accumulation groups; one
            # DVE eviction (psum + per-partition bias -> bf16)
            ps = ps_big.tile([P, 1024], f32, tag="mm", name="mm")
            for half in range(2):
                for ci in range(C):
                    nc.tensor.matmul(
                        ps[:, half * 512:(half + 1) * 512],
                        w[:, ci, co * P:(co + 1) * P],
                        xnT[:, ci, (n0 + half) * 512:(n0 + half + 1) * 512],
                        start=(ci == 0), stop=(ci == C - 1),
                    )
            nc.vector.tensor_scalar(
                dstT[:, co, n0 * 512:(n0 + 2) * 512], ps[:],
                bias_c[:, co:co + 1], None, OP.add,
            )

        for co in range(C):  # output chunk: heads 2co, 2co+1
            proj_qk(w_q, qT, bq_c, co, 0, 2)
            proj_qk(w_k, kT, bk_c, co, 0, 2)
            proj_qk(w_k, kT, bk_c, co, 2, 2)
        # V: [tokens, HE] = sum_c xnT[:,c,tok-tile].T @ wv[:,c,:]
        for t in range(NKT):
            if CFG["v_pool"] == "att":
                ps = ps_att.tile([P, 512], f32, tag="att", name="vps")
            else:
                ps = ps_big.tile([P, 512], f32, tag="mm", name="vps")
            for ci in range(C):
                nc.tensor.matmul(
                    ps[:],
                    xnT[:, ci, t * P:(t + 1) * P],
                    w_v[:, ci, :],
                    start=(ci == 0), stop=(ci == C - 1),
                )
            vt = v_sb[:, t, :].rearrange("p (h e) -> p h e", h=H)
            nc.vector.tensor_tensor(
                vt[:, :, 0:E],
                ps[:].rearrange("p (h e) -> p h e", h=H),
                bv_b[:].rearrange("p (h e) -> p h e", h=H),
                OP.add,
            )
            nc.vector.memset(vt[:, :, E:EA], 1.0)

        # ---- phase C: attention per head ----------------------------
        for h in range(H):
            ch, off = h // 2, (h % 2) * E
            att = ps_att.tile([EA, SQ], f32, tag="att", name="att")

            def att_mm(kt, ex):
                for n in range(SQ // 512):
                    nc.tensor.matmul(
                        att[:, n * 512:(n + 1) * 512],
                        v_sb[:, kt, h * EA:(h + 1) * EA],
                        ex[:, n * 512:(n + 1) * 512],
                        start=(kt == 0), stop=(kt == NKT - 1),
                    )

            pending = None
            for kt in range(NKT):
                if CFG["scs_alt"] and kt % 2 == 1:
                    scs = ps_att.tile([P, SQ], f32, tag="att", name="scs")
                else:
                    scs = ps_big.tile([P, SQ], f32, tag="mm", name="mm")
                for n in range(SQ // 512):
                    nc.tensor.matmul(
                        scs[:, n * 512:(n + 1) * 512],
                        kT[off:off + E, ch, kt * P:(kt + 1) * P],
                        qT[off:off + E, ch, n * 512:(n + 1) * 512],
                        start=True, stop=True,
                    )
                ex = pexp.tile([P, SQ], bf16, tag="ex", name="ex")
                m = CFG["dve_exp_mod"]
                if m and kt % m == m - 1:
                    nc.vector._custom_dve(
                        xp_op, out=scs[:], in0=scs[:], s0=EC1, s1=EC2, imm2=EC3
                    )
                    nc.vector._custom_dve(xs_op, out=ex[:], in0=scs[:])
                else:
                    nc.scalar.activation(
                        ex[:], scs[:], AF.Exp, scale=float(SCALE)
                    )
                if CFG["swpipe"]:
                    if pending is not None:
                        att_mm(kt - 1, pending)
                    pending = ex
                else:
                    att_mm(kt, ex)
            if CFG["swpipe"]:
                att_mm(NKT - 1, pending)
            # normalize: att[:E] / att[E]; the recip row sits at base
            # partition 0 (matmul requires lhsT/rhs partition bases match)
            att_un = ptmp.tile([E, SQ], f32, tag="tmp", name="tmp")
            ne = nc.gpsimd if CFG["norm_eng"] == "gpsimd" else nc.vector
            ne.tensor_copy(att_un[:], att[0:E, :])
            rrt = prr.tile([1, SQ], f32r, tag="rr", name="rr")
            rr = rrt[:]
            with nc.allow_low_precision(
                reason="softmax denom recip rounded to f32r for the "
                "broadcast matmul; ~1e-6 relative"
            ):
                nc.vector.reciprocal(rr, att[E:EA, :])
            bc = ps_big.tile([E, SQ], f32, tag="mm", name="mm")
            for n in range(SQ // 512):
                nc.tensor.matmul(
                    bc[:, n * 512:(n + 1) * 512], ones64[:],
                    rr[:, n * 512:(n + 1) * 512],
                    start=True, stop=True,
                )
            ne.tensor_tensor(
                attnT[off:off + E, ch, :], att_un[:], bc[:], OP.mult
            )

        # ---- phase D: projection + residual + norm2 -----------------
        for qt in range(NQT):
            ps = ps_big.tile([P, 512], f32, tag="mm", name="mm")
            for c in range(C):
                nc.tensor.matmul(
                    ps[:],
                    attnT[:, c, qt * P:(qt + 1) * P],
                    w_p[:, c, :],
                    start=(c == 0), stop=(c == C - 1),
                )
            xq = px.tile([P, D], f32, tag="x", name="x")
            nc.sync.dma_start(xq[:], xqbp[:, qt, :])
            nc.vector.tensor_tensor(x1_sb[:, qt, :], ps[:], xq[:], OP.add)
            mv, rstd = norm_stats(x1_sb[:, qt, :])
            x1n = pxn.tile([P, D], bf16, tag="xn", name="xn")
            nc.vector.tensor_scalar(
                x1n[:], x1_sb[:, qt, :], mv[:, 0:1], rstd[:], OP.subtract, OP.mult
            )
            transpose_into(x1nT, x1n[:], qt)

        # ---- phase E: FFN1 + gelu -----------------------------------
        for n in range(SQ // 512):
            for fc in range(FC):
                ps = ps_big.tile([P, 512], f32, tag="mm", name="mm")
                for c in range(C):
                    nc.tensor.matmul(
                        ps[:],
                        w_1[:, c, fc * P:(fc + 1) * P],
                        x1nT[:, c, n * 512:(n + 1) * 512],
                        start=(c == 0), stop=(c == C - 1),
                    )
                nc.scalar.activation(
                    hT[:, fc, n * 512:(n + 1) * 512], ps[:],
                    AF.Gelu, bias=b1_c[:, fc:fc + 1],
                )

        # ---- phase F: FFN2 + gelu + residual ------------------------
        for qt in range(NQT):
            ps = ps_big.tile([P, 512], f32, tag="mm", name="mm")
            for fc in range(FC):
                nc.tensor.matmul(
                    ps[:],
                    hT[:, fc, qt * P:(qt + 1) * P],
                    w_2[:, fc, :],
                    start=(fc == 0), stop=(fc == FC - 1),
                )
            pre2 = ptmp.tile([P, D], f32, tag="tmp", name="tmp")
            nc.vector.tensor_tensor(pre2[:], ps[:], b2_b[:], OP.add)
            g2 = ptmp.tile([P, D], f32, tag="tmp", name="tmp")
            nc.scalar.activation(g2[:], pre2[:], AF.Gelu)
            yt = ptmp.tile([P, D], f32, tag="tmp", name="tmp")
            nc.vector.tensor_tensor(yt[:], g2[:], x1_sb[:, qt, :], OP.add)
            nc.sync.dma_start(y_out[:, qt, :], yt[:])

    nc.compile()
    return nc


def _pack_pmajor(a, ntiles):
    """[ntiles*128, W] -> [128, ntiles, W] with tile t, partition p = row t*128+p."""
    return np.ascontiguousarray(a.reshape(ntiles, P, -1).transpose(1, 0, 2))


def _prep_shared(Wq, bq, Wk, bk, Wv, bv, Wp, gamma1, beta1, gamma2, beta2,
                 W1, b1, W2, b2):
    g1 = np.asarray(gamma1, np.float64)
    be1 = np.asarray(beta1, np.float64)
    g2 = np.asarray(gamma2, np.float64)
    be2 = np.asarray(beta2, np.float64)

    def headcat(w):  # [H, D, E] -> [D, H*E]
        return np.ascontiguousarray(
            np.transpose(np.asarray(w, np.float64), (1, 0, 2)).reshape(D, H * E)
        )

    out = {}
    for name, w, b in [("q", Wq, bq), ("k", Wk, bk)]:
        wa = headcat(w)
        beff = np.asarray(b, np.float64).reshape(-1) + be1 @ wa
        wag = wa * g1[:, None]
        out["w" + name] = _pack_pmajor(wag, C).astype(BF16)
        out["b" + name + "_c"] = np.ascontiguousarray(
            beff.reshape(C, P).T
        ).astype(np.float32)
    wv_a = headcat(Wv)
    bv_eff = np.asarray(bv, np.float64).reshape(-1) + be1 @ wv_a
    out["wv"] = _pack_pmajor(wv_a * g1[:, None], C).astype(BF16)
    out["bv_b"] = np.ascontiguousarray(
        np.broadcast_to(bv_eff.astype(np.float32), (P, H * E))
    )
    out["wp"] = _pack_pmajor(np.asarray(Wp, np.float64), C).astype(BF16)
    w1_a = np.asarray(W1, np.float64)
    b1_eff = np.asarray(b1, np.float64) + be2 @ w1_a
    out["w1"] = _pack_pmajor(w1_a * g2[:, None], C).astype(BF16)
    out["b1_c"] = np.ascontiguousarray(b1_eff.reshape(FC, P).T).astype(np.float32)
    out["w2"] = _pack_pmajor(np.asarray(W2, np.float64), FC).astype(BF16)
    out["b2_b"] = np.ascontiguousarray(
        np.broadcast_to(np.asarray(b2, np.float32), (P, D))
    )
    out["ident"] = np.eye(P, dtype=BF16)
    out["ones64"] = np.ones((1, E), dtype=np.float32)
    return out


def _gather(results):
    y = np.empty((B, S, D), np.float32)
    for core in range(8):
        b_idx, half = core // 2, core % 2
        yp = np.asarray(results[core]["y_out"], np.float32)
        y[b_idx, half * SQ:(half + 1) * SQ] = (
            yp.transpose(1, 0, 2).reshape(SQ, D)
        )
    return y.reshape(B, S, D, 1, 1)


def kernel(x, Wq, bq, Wk, bk, Wv, bv, Wp, bp, gamma1, beta1, gamma2, beta2,
           W1, b1, W2, b2):
    from concourse.bass_utils import run_bass_kernel_spmd

    if "nc" not in _CACHE:
        _CACHE["nc"] = _build_program()
    nc = _CACHE["nc"]

    weights = dict(
        Wq=Wq, bq=bq, Wk=Wk, bk=bk, Wv=Wv, bv=bv, Wp=Wp,
        gamma1=gamma1, beta1=beta1, gamma2=gamma2, beta2=beta2,
        W1=W1, b1=b1, W2=W2, b2=b2,
    )
    x_flat = np.asarray(x, np.float32).reshape(B, S, D)
    shared = _prep_shared(**weights)
    bp_a = np.asarray(bp, np.float32)
    in_maps = []
    for core in range(8):
        b_idx, half = core // 2, core % 2
        xo = np.roll(x_flat[b_idx], -half * SQ, axis=0)
        m = dict(shared)
        m["x_all"] = _pack_pmajor(xo, NKT)
        m["xqbp"] = _pack_pmajor(xo[:SQ] + bp_a[None, :], NQT)
        in_maps.append(m)

    res = run_bass_kernel_spmd(nc, in_maps, core_ids=list(range(8)))
    return _gather(res.results)


# revision 51
# speedup vs baseline: 1.1017x; 1.1017x over previous
"""Trainium2 Bass kernel for a dense transformer encoder layer.

Model dims: B=4, S=2048, D=512, H=8 heads, E=64 head dim, F=2048 ffn dim.

Sharding: 8 cores, core c -> (batch b = c//2, sequence half = c%2).
Each core receives its batch's full 2048 tokens (reordered so the core's
1024 query rows come first) and computes the full layer for its 1024
query tokens; K/V are computed for all 2048 tokens on-core, so no
cross-core communication is needed (softmax over keys is permutation
invariant, so the sequence reorder is harmless).

Layer math on one core (q = 1024 query tokens, k = 2048 kv tokens):
  norm1 (layernorm, Bessel std) -> x_norm^T [D, k] bf16 (PE transposes)
  Q^T/K^T = W_{q,k}^T x_norm^T (+bias, per-partition)   [HE, q|k]
  V      = x_norm W_v (+bias) stored [k, H*(E+1)] with a ones column per
           head so the attention GEMM also produces the softmax row sums
  scores^T = K_h Q_h^T (K=64 contraction), exp on ScalarE (scale=1/8)
  att^T[e,q](+sums row) = V_aug^T exp^T accumulated over k tiles
  normalize: recip(sums) -> K=1 matmul broadcast -> multiply
  att_out = att_norm^T^T Wp; x1 = att_out + x + bp; norm2; FFN with
  exact Gelu on both FFN outputs; y = gelu2 + x1.

gamma/beta of both norms are folded into the adjacent GEMM weights on the
host.  All GEMMs run in bf16 with fp32 PSUM accumulation.
"""

import numpy as np
import ml_dtypes

B, S, D, H, E, F = 4, 2048, 512, 8, 64, 2048
P = 128
SQ = S // 2          # query tokens per core
NQT = SQ // P        # 8 query 128-tiles
NKT = S // P         # 16 kv 128-tiles
C = D // P           # 4 chunks of the model dim
FC = F // P          # 16 chunks of the ffn dim
EA = E + 1           # head dim + ones column
SCALE = 1.0 / np.sqrt(E)
BESSEL = D / (D - 1.0)  # ddof=1 correction on variance

BF16 = ml_dtypes.bfloat16

# exp(s/8) = p(s)^32 with p a deg-3 fit of exp(s/256) over |s/256|<=0.23;
# runs on the Vector engine to offload softmax exp from ScalarE
EC1, EC2, EC3 = 3.90639966e-03, 7.65718235e-06, 9.89457506e-09

_CACHE = {}

# tuning knobs (swept via t_sweep.py)
CFG = {
    "ps_big_bufs": 2,    # scores/proj/ffn psum slots (2 banks each)
    "ps_att_bufs": 2,    # att accumulator slots (2 banks each)
    "v_pool": "att",     # which pool V-projection psums come from
    "tr_pool": "att",    # which pool transpose psums come from
    "dve_exp_mod": 0,    # kt % mod == mod-1 goes to DVE; 0 = ACT only
    "swpipe": True,      # delay att GEMMs one kt behind exp
    "scs_alt": True,     # alternate score tiles between psum pools
    "norm_eng": "dve",   # engine for the softmax-normalize copy/mult
    "order": "0011",
    "px_bufs": 5,
    "pxn_bufs": 3,
    "ptmp_bufs": 2,
    "pexp_bufs": 3,
}


def _register_dve_exp():
    import numpy as _np
    from concourse import dve_ops as DO
    from concourse.dve_spec import Spec, Src0, C0, C1, C2, One, sq, lower
    from concourse.dve_ops import has_src1
    from concourse.dve_uop import DveOpSpec

    if "EXP32_POLY_ANT" in DO._SUB_OPCODE_FOR_NAME:
        by = {op.name: op for op in DO.OPS}
        return by["EXP32_POLY_ANT"], by["EXP32_SQ_ANT"]

    s = Src0
    specs = [
        ("EXP32_POLY_ANT", Spec(
            body=((s * C2 + C1) * s + C0) * s + One,
            reference=lambda in0, in1, s0, s1, imm2: (
                (in0 * imm2 + s1) * in0 + s0) * in0 + 1.0)),
        ("EXP32_SQ_ANT", Spec(
            body=sq(sq(sq(sq(sq(s))))),
            reference=lambda in0, in1, s0, s1, imm2: (
                in0.astype(_np.float64) ** 32))),
    ]
    ops = []
    for name, spec in specs:
        op = DO.DveOp(name, spec, subdim=False, uops_sha={})
        DO.OPS.append(op)
        DO._SUB_OPCODE_FOR_NAME[name] = DO._CUSTOM_DVE_ROW_BASE + len(DO.OPS) - 1
        DO.CUSTOM_DVE_SPECS[name] = spec
        so = DveOpSpec(name=name, opcode=DO.get_dve_sub_opcode(name),
                       uops=lower(spec, ver="v3"), rd1_en=has_src1(spec))
        op.uops_sha["v3"] = so.sha("v3")
        ops.append(op)
    assert max(DO._SUB_OPCODE_FOR_NAME.values()) < 0x20
    return ops[0], ops[1]


def _build_program():
    """Build (and cache) the SPMD Bass program. Returns (nc, names)."""
    from contextlib import ExitStack

    import concourse.bass as bass
    import concourse.mybir as mybir
    import concourse.tile as tile
    from concourse import bacc

    f32 = mybir.dt.float32
    bf16 = mybir.dt.bfloat16
    AF = mybir.ActivationFunctionType
    OP = mybir.AluOpType

    xp_op, xs_op = _register_dve_exp()

    nc = bacc.Bacc(None, target_bir_lowering=False)

    # ---- DRAM I/O ----------------------------------------------------
    x_all = nc.dram_tensor("x_all", [P, NKT, D], f32, kind="ExternalInput")
    xqbp = nc.dram_tensor("xqbp", [P, NQT, D], f32, kind="ExternalInput")
    wq_d = nc.dram_tensor("wq", [P, C, H * E], bf16, kind="ExternalInput")
    wk_d = nc.dram_tensor("wk", [P, C, H * E], bf16, kind="ExternalInput")
    wv_d = nc.dram_tensor("wv", [P, C, H * E], bf16, kind="ExternalInput")
    wp_d = nc.dram_tensor("wp", [P, C, D], bf16, kind="ExternalInput")
    w1_d = nc.dram_tensor("w1", [P, C, F], bf16, kind="ExternalInput")
    w2_d = nc.dram_tensor("w2", [P, FC, D], bf16, kind="ExternalInput")
    bq_d = nc.dram_tensor("bq_c", [P, C], f32, kind="ExternalInput")
    bk_d = nc.dram_tensor("bk_c", [P, C], f32, kind="ExternalInput")
    bv_d = nc.dram_tensor("bv_b", [P, H * E], f32, kind="ExternalInput")
    b1_d = nc.dram_tensor("b1_c", [P, FC], f32, kind="ExternalInput")
    b2_d = nc.dram_tensor("b2_b", [P, D], f32, kind="ExternalInput")
    id_d = nc.dram_tensor("ident", [P, P], bf16, kind="ExternalInput")
    f32r = mybir.dt.float32r
    on_d = nc.dram_tensor("ones64", [1, E], f32r, kind="ExternalInput")
    y_out = nc.dram_tensor("y_out", [P, NQT, D], f32, kind="ExternalOutput")

    with tile.TileContext(nc) as tc, ExitStack() as ctx:
        pers = ctx.enter_context(tc.tile_pool(name="pers", bufs=1))
        px = ctx.enter_context(tc.tile_pool(name="px", bufs=CFG["px_bufs"]))
        pxn = ctx.enter_context(tc.tile_pool(name="pxn", bufs=CFG["pxn_bufs"]))
        pexp = ctx.enter_context(tc.tile_pool(name="pexp", bufs=CFG["pexp_bufs"]))
        ptmp = ctx.enter_context(tc.tile_pool(name="ptmp", bufs=CFG["ptmp_bufs"]))
        pst = ctx.enter_context(tc.tile_pool(name="pst", bufs=8))
        prr = ctx.enter_context(tc.tile_pool(name="prr", bufs=1))
        ps_big = ctx.enter_context(tc.tile_pool(name="ps_big", bufs=CFG["ps_big_bufs"], space="PSUM"))
        ps_att = ctx.enter_context(tc.tile_pool(name="ps_att", bufs=CFG["ps_att_bufs"], space="PSUM"))

        # ---- persistent SBUF tensors --------------------------------
        def pt(shape, dt, tag):
            return pers.tile(shape, dt, tag=tag, name=tag)

        w_q = pt([P, C, H * E], bf16, "w_q")
        w_k = pt([P, C, H * E], bf16, "w_k")
        w_v = pt([P, C, H * E], bf16, "w_v")
        w_p = pt([P, C, D], bf16, "w_p")
        w_1 = pt([P, C, F], bf16, "w_1")
        w_2 = pt([P, FC, D], bf16, "w_2")
        bq_c = pt([P, C], f32, "bq_c")
        bk_c = pt([P, C], f32, "bk_c")
        bv_b = pt([P, H * E], f32, "bv_b")
        b1_c = pt([P, FC], f32, "b1_c")
        b2_b = pt([P, D], f32, "b2_b")
        ident = pt([P, P], bf16, "ident")
        ones64 = pt([1, E], f32r, "ones64")
        xnT = pt([P, C, S], bf16, "xnT")
        qT = pt([P, C, SQ], bf16, "qT")
        kT = pt([P, C, S], bf16, "kT")
        v_sb = pt([P, NKT, H * EA], bf16, "v_sb")
        attnT = pt([P, C, SQ], bf16, "attnT")
        x1_sb = pt([P, NQT, D], f32, "x1_sb")
        x1nT = pt([P, C, SQ], bf16, "x1nT")
        hT = pt([P, FC, SQ], bf16, "hT")

        for dst, src in [
            (ident, id_d), (w_q, wq_d), (w_k, wk_d), (w_v, wv_d),
            (bq_c, bq_d), (bk_c, bk_d), (bv_b, bv_d), (ones64, on_d),
            (w_p, wp_d), (b1_c, b1_d), (b2_b, b2_d),
            (w_1, w1_d), (w_2, w2_d),
        ]:
            nc.sync.dma_start(dst[:], src[:])

        # ---- helper: layernorm stats -> (mean, rstd) ----------------
        def norm_stats(xt):
            st6 = pst.tile([P, 6], f32, tag="st6", name="st6")
            nc.vector.bn_stats(st6[:], xt)
            mv = pst.tile([P, 2], f32, tag="mv", name="mv")
            nc.vector.bn_aggr(mv[:], st6[:])
            std = pst.tile([P, 1], f32, tag="std", name="std")
            nc.scalar.activation(std[:], mv[:, 1:2], AF.Sqrt, scale=BESSEL)
            rstd = pst.tile([P, 1], f32, tag="rstd", name="rstd")
            nc.vector.reciprocal(rstd[:], std[:])
            return mv, rstd

        # transpose a [P, D] bf16 tile into dstT[:, :, tcol*P : +P]
        def transpose_into(dstT, xn, tcol):
            if CFG["tr_pool"] == "att":
                ps = ps_att.tile([P, 512], bf16, tag="att", name="tr")
            else:
                ps = ps_big.tile([P, 512], bf16, tag="mm", name="tr")
            for c in range(C):
                nc.tensor.transpose(
                    ps[:, c * P:(c + 1) * P], xn[:, c * P:(c + 1) * P], ident[:]
                )
            nc.scalar.copy(
                dstT[:, :, tcol * P:(tcol + 1) * P],
                ps[:].rearrange("p (c j) -> p c j", c=C),
            )

        # ---- phase A: norm1 + transpose ------------------------------
        for t in range(NKT):
            xt = px.tile([P, D], f32, tag="x", name="x")
            nc.gpsimd.dma_start(xt[:], x_all[:, t, :])
            mv, rstd = norm_stats(xt[:])
            xn = pxn.tile([P, D], bf16, tag="xn", name="xn")
            nc.gpsimd.tensor_scalar(
                xn[:], xt[:], mv[:, 0:1], rstd[:], OP.subtract, OP.mult
            )
            transpose_into(xnT, xn[:], t)
            # V for tile t needs only this tile's xnT columns -> emit now
            ps = ps_att.tile([P, 512], f32, tag="att", name="vps")
            for ci in range(C):
                nc.tensor.matmul(
                    ps[:],
                    xnT[:, ci, t * P:(t + 1) * P],
                    w_v[:, ci, :],
                    start=(ci == 0), stop=(ci == C - 1),
                )
            vt = v_sb[:, t, :].rearrange("p (h e) -> p h e", h=H)
            nc.vector.tensor_tensor(
                vt[:, :, 0:E],
                ps[:].rearrange("p (h e) -> p h e", h=H),
                bv_b[:].rearrange("p (h e) -> p h e", h=H),
                OP.add,
            )
            nc.vector.memset(vt[:, :, E:EA], 1.0)

        # ---- phase B: QKV projections -------------------------------
        # Q^T / K^T: [HE, tokens] = sum_c W[:,c,:].T @ xnT[:,c,:]
        def proj_qk(w, dstT, bias_c, co, n0, ntiles):
            # [128, 1024] psum = two 512-wide accumulation groups; one
            # DVE eviction (psum + per-partition bias -> bf16)
            ps = ps_big.tile([P, 1024], f32, tag="mm", name="mm")
            for half in range(2):
                for ci in range(C):
                    nc.tensor.matmul(
                        ps[:, half * 512:(half + 1) * 512],
                        w[:, ci, co * P:(co + 1) * P],
                        xnT[:, ci, (n0 + half) * 512:(n0 + half + 1) * 512],
                        start=(ci == 0), stop=(ci == C - 1),
                    )
            nc.vector.tensor_scalar(
                dstT[:, co, n0 * 512:(n0 + 2) * 512], ps[:],
                bias_c[:, co:co + 1], None, OP.add,
            )

        # n-outer: the n=0 projections only need token tiles 0-3
        proj_qk(w_q, qT, bq_c, 0, 0, 2)
        proj_qk(w_k, kT, bk_c, 0, 0, 2)
        for co in range(1, C):
            proj_qk(w_q, qT, bq_c, co, 0, 2)
            proj_qk(w_k, kT, bk_c, co, 0, 2)
        for co in range(C):
            proj_qk(w_k, kT, bk_c, co, 2, 2)

        # ---- phase C: attention per head ----------------------------
        # The head-boundary normalize is split: the DVE part (recip +
        # copy) runs right after the last att GEMM, but the broadcast
        # matmul + final multiply are deferred into the NEXT head's kt
        # loop, so the PE never stalls in program order waiting on DVE.
        def finish_head(h, att_un, rr):
            ch, off = h // 2, (h % 2) * E
            bc = ps_big.tile([E, SQ], f32, tag="mm", name="mm")
            for n in range(SQ // 512):
                nc.tensor.matmul(
                    bc[:, n * 512:(n + 1) * 512], ones64[:],
                    rr[:, n * 512:(n + 1) * 512],
                    start=True, stop=True,
                )
            nc.vector.tensor_tensor(
                attnT[off:off + E, ch, :], att_un[:], bc[:], OP.mult
            )

        deferred = None
        for h in range(H):
            ch, off = h // 2, (h % 2) * E
            att = ps_att.tile([EA, SQ], f32, tag="att", name="att")

            def att_mm(kt, ex):
                for n in range(SQ // 512):
                    nc.tensor.matmul(
                        att[:, n * 512:(n + 1) * 512],
                        v_sb[:, kt, h * EA:(h + 1) * EA],
                        ex[:, n * 512:(n + 1) * 512],
                        start=(kt == 0), stop=(kt == NKT - 1),
                    )

            pending = None
            for kt in range(NKT):
                if CFG["scs_alt"] and kt % 2 == 1:
                    scs = ps_att.tile([P, SQ], f32, tag="att", name="scs")
                else:
                    scs = ps_big.tile([P, SQ], f32, tag="mm", name="mm")
                for n in range(SQ // 512):
                    nc.tensor.matmul(
                        scs[:, n * 512:(n + 1) * 512],
                        kT[off:off + E, ch, kt * P:(kt + 1) * P],
                        qT[off:off + E, ch, n * 512:(n + 1) * 512],
                        start=True, stop=True,
                    )
                ex = pexp.tile([P, SQ], bf16, tag="ex", name="ex")
                m = CFG["dve_exp_mod"]
                if m and kt % m == m - 1:
                    nc.vector._custom_dve(
                        xp_op, out=scs[:], in0=scs[:], s0=EC1, s1=EC2, imm2=EC3
                    )
                    nc.vector._custom_dve(xs_op, out=ex[:], in0=scs[:])
                else:
                    nc.scalar.activation(
                        ex[:], scs[:], AF.Exp, scale=float(SCALE)
                    )
                if pending is not None:
                    att_mm(kt - 1, pending)
                pending = ex
                if kt == 2 and deferred is not None:
                    finish_head(*deferred)
                    deferred = None
            att_mm(NKT - 1, pending)
            # immediate DVE part: recip first (bcast only needs this),
            # then the att_un eviction copy
            rrt = prr.tile([1, SQ], f32r, tag="rr", name="rr")
            with nc.allow_low_precision(
                reason="softmax denom recip rounded to f32r for the "
                "broadcast matmul; ~1e-6 relative"
            ):
                nc.vector.reciprocal(rrt[:], att[E:EA, :])
            att_un = ptmp.tile([E, SQ], f32, tag="tmp", name="tmp")
            nc.vector.tensor_copy(att_un[:], att[0:E, :])
            deferred = (h, att_un, rrt[:])
        finish_head(*deferred)

        # ---- phase D: projection + residual + norm2 -----------------
        # pipelined one qt deep: the PE transposes of qt wait on a DVE
        # stats chain, so qt+1's projection matmuls are emitted first
        d_pend = None
        for qt in range(NQT):
            ps = ps_big.tile([P, 512], f32, tag="mm", name="mm")
            for c in range(C):
                nc.tensor.matmul(
                    ps[:],
                    attnT[:, c, qt * P:(qt + 1) * P],
                    w_p[:, c, :],
                    start=(c == 0), stop=(c == C - 1),
                )
            xq = px.tile([P, D], f32, tag="x", name="x")
            nc.sync.dma_start(xq[:], xqbp[:, qt, :])
            nc.vector.tensor_tensor(x1_sb[:, qt, :], ps[:], xq[:], OP.add)
            mv, rstd = norm_stats(x1_sb[:, qt, :])
            x1n = pxn.tile([P, D], bf16, tag="xn", name="xn")
            nc.gpsimd.tensor_scalar(
                x1n[:], x1_sb[:, qt, :], mv[:, 0:1], rstd[:], OP.subtract, OP.mult
            )
            if d_pend is not None:
                transpose_into(x1nT, d_pend[1], d_pend[0])
            d_pend = (qt, x1n[:])
        transpose_into(x1nT, d_pend[1], d_pend[0])

        # ---- phase E: FFN1 + gelu -----------------------------------
        for n in range(SQ // 512):
            for fc in range(FC):
                ps = ps_big.tile([P, 512], f32, tag="mm", name="mm")
                for c in range(C):
                    nc.tensor.matmul(
                        ps[:],
                        w_1[:, c, fc * P:(fc + 1) * P],
                        x1nT[:, c, n * 512:(n + 1) * 512],
                        start=(c == 0), stop=(c == C - 1),
                    )
                nc.scalar.activation(
                    hT[:, fc, n * 512:(n + 1) * 512], ps[:],
                    AF.Gelu, bias=b1_c[:, fc:fc + 1],
                )

        # ---- phase F: FFN2 + gelu + residual ------------------------
        for qt in range(NQT):
            ps = ps_big.tile([P, 512], f32, tag="mm", name="mm")
            for fc in range(FC):
                nc.tensor.matmul(
                    ps[:],
                    hT[:, fc, qt * P:(qt + 1) * P],
                    w_2[:, fc, :],
                    start=(fc == 0), stop=(fc == FC - 1),
                )
            pre2 = ptmp.tile([P, D], f32, tag="tmp", name="tmp")
            nc.vector.tensor_tensor(pre2[:], ps[:], b2_b[:], OP.add)
            g2 = ptmp.tile([P, D], f32, tag="tmp", name="tmp")
            nc.scalar.activation(g2[:], pre2[:], AF.Gelu)
            yt = ptmp.tile([P, D], f32, tag="tmp", name="tmp")
            nc.vector.tensor_tensor(yt[:], g2[:], x1_sb[:, qt, :], OP.add)
            nc.sync.dma_start(y_out[:, qt, :], yt[:])

    nc.compile()
    return nc


def _pack_pmajor(a, ntiles):
    """[ntiles*128, W] -> [128, ntiles, W] with tile t, partition p = row t*128+p."""
    return np.ascontiguousarray(a.reshape(ntiles, P, -1).transpose(1, 0, 2))


def _prep_shared(Wq, bq, Wk, bk, Wv, bv, Wp, gamma1, beta1, gamma2, beta2,
                 W1, b1, W2, b2):
    g1 = np.asarray(gamma1, np.float64)
    be1 = np.asarray(beta1, np.float64)
    g2 = np.asarray(gamma2, np.float64)
    be2 = np.asarray(beta2, np.float64)

    def headcat(w):  # [H, D, E] -> [D, H*E]
        return np.ascontiguousarray(
            np.transpose(np.asarray(w, np.float64), (1, 0, 2)).reshape(D, H * E)
        )

    out = {}
    for name, w, b in [("q", Wq, bq), ("k", Wk, bk)]:
        wa = headcat(w)
        beff = np.asarray(b, np.float64).reshape(-1) + be1 @ wa
        wag = wa * g1[:, None]
        out["w" + name] = _pack_pmajor(wag, C).astype(BF16)
        out["b" + name + "_c"] = np.ascontiguousarray(
            beff.reshape(C, P).T
        ).astype(np.float32)
    wv_a = headcat(Wv)
    bv_eff = np.asarray(bv, np.float64).reshape(-1) + be1 @ wv_a
    out["wv"] = _pack_pmajor(wv_a * g1[:, None], C).astype(BF16)
    out["bv_b"] = np.ascontiguousarray(
        np.broadcast_to(bv_eff.astype(np.float32), (P, H * E))
    )
    out["wp"] = _pack_pmajor(np.asarray(Wp, np.float64), C).astype(BF16)
    w1_a = np.asarray(W1, np.float64)
    b1_eff = np.asarray(b1, np.float64) + be2 @ w1_a
    out["w1"] = _pack_pmajor(w1_a * g2[:, None], C).astype(BF16)
    out["b1_c"] = np.ascontiguousarray(b1_eff.reshape(FC, P).T).astype(np.float32)
    out["w2"] = _pack_pmajor(np.asarray(W2, np.float64), FC).astype(BF16)
    out["b2_b"] = np.ascontiguousarray(
        np.broadcast_to(np.asarray(b2, np.float32), (P, D))
    )
    out["ident"] = np.eye(P, dtype=BF16)
    out["ones64"] = np.ones((1, E), dtype=np.float32)
    return out


def _gather(results):
    y = np.empty((B, S, D), np.float32)
    for core in range(8):
        b_idx, half = core // 2, core % 2
        yp = np.asarray(results[core]["y_out"], np.float32)
        y[b_idx, half * SQ:(half + 1) * SQ] = (
            yp.transpose(1, 0, 2).reshape(SQ, D)
        )
    return y.reshape(B, S, D, 1, 1)


def kernel(x, Wq, bq, Wk, bk, Wv, bv, Wp, bp, gamma1, beta1, gamma2, beta2,
           W1, b1, W2, b2):
    from concourse.bass_utils import run_bass_kernel_spmd

    if "nc" not in _CACHE:
        _CACHE["nc"] = _build_program()
    nc = _CACHE["nc"]

    weights = dict(
        Wq=Wq, bq=bq, Wk=Wk, bk=bk, Wv=Wv, bv=bv, Wp=Wp,
        gamma1=gamma1, beta1=beta1, gamma2=gamma2, beta2=beta2,
        W1=W1, b1=b1, W2=W2, b2=b2,
    )
    x_flat = np.asarray(x, np.float32).reshape(B, S, D)
    shared = _prep_shared(**weights)
    bp_a = np.asarray(bp, np.float32)
    in_maps = []
    for core in range(8):
        b_idx, half = core // 2, core % 2
        xo = np.roll(x_flat[b_idx], -half * SQ, axis=0)
        m = dict(shared)
        m["x_all"] = _pack_pmajor(xo, NKT)
        m["xqbp"] = _pack_pmajor(xo[:SQ] + bp_a[None, :], NQT)
        in_maps.append(m)

    res = run_bass_kernel_spmd(nc, in_maps, core_ids=list(range(8)))
    return _gather(res.results)


# revision 52
# speedup vs baseline: 1.1024x; 1.0006x over previous
"""Trainium2 Bass kernel for a dense transformer encoder layer.

Model dims: B=4, S=2048, D=512, H=8 heads, E=64 head dim, F=2048 ffn dim.

Sharding: 8 cores, core c -> (batch b = c//2, sequence half = c%2).
Each core receives its batch's full 2048 tokens (reordered so the core's
1024 query rows come first) and computes the full layer for its 1024
query tokens; K/V are computed for all 2048 tokens on-core, so no
cross-core communication is needed (softmax over keys is permutation
invariant, so the sequence reorder is harmless).

Layer math on one core (q = 1024 query tokens, k = 2048 kv tokens):
  norm1 (layernorm, Bessel std) -> x_norm^T [D, k] bf16 (PE transposes)
  Q^T/K^T = W_{q,k}^T x_norm^T (+bias, per-partition)   [HE, q|k]
  V      = x_norm W_v (+bias) stored [k, H*(E+1)] with a ones column per
           head so the attention GEMM also produces the softmax row sums
  scores^T = K_h Q_h^T (K=64 contraction), exp on ScalarE (scale=1/8)
  att^T[e,q](+sums row) = V_aug^T exp^T accumulated over k tiles
  normalize: recip(sums) -> K=1 matmul broadcast -> multiply
  att_out = att_norm^T^T Wp; x1 = att_out + x + bp; norm2; FFN with
  exact Gelu on both FFN outputs; y = gelu2 + x1.

gamma/beta of both norms are folded into the adjacent GEMM weights on the
host.  All GEMMs run in bf16 with fp32 PSUM accumulation.
"""

import numpy as np
import ml_dtypes

B, S, D, H, E, F = 4, 2048, 512, 8, 64, 2048
P = 128
SQ = S // 2          # query tokens per core
NQT = SQ // P        # 8 query 128-tiles
NKT = S // P         # 16 kv 128-tiles
C = D // P           # 4 chunks of the model dim
FC = F // P          # 16 chunks of the ffn dim
EA = E + 1           # head dim + ones column
SCALE = 1.0 / np.sqrt(E)
BESSEL = D / (D - 1.0)  # ddof=1 correction on variance

BF16 = ml_dtypes.bfloat16

# exp(s/8) = p(s)^32 with p a deg-3 fit of exp(s/256) over |s/256|<=0.23;
# runs on the Vector engine to offload softmax exp from ScalarE
EC1, EC2, EC3 = 3.90639966e-03, 7.65718235e-06, 9.89457506e-09

_CACHE = {}

# tuning knobs (swept via t_sweep.py)
CFG = {
    "ps_big_bufs": 2,    # scores/proj/ffn psum slots (2 banks each)
    "ps_att_bufs": 2,    # att accumulator slots (2 banks each)
    "v_pool": "att",     # which pool V-projection psums come from
    "tr_pool": "att",    # which pool transpose psums come from
    "dve_exp_mod": 0,    # kt % mod == mod-1 goes to DVE; 0 = ACT only
    "swpipe": True,      # delay att GEMMs one kt behind exp
    "scs_alt": True,     # alternate score tiles between psum pools
    "norm_eng": "dve",   # engine for the softmax-normalize copy/mult
    "order": "0011",
    "px_bufs": 4,
    "pxn_bufs": 3,
    "ptmp_bufs": 2,
    "pexp_bufs": 5,
}


def _register_dve_exp():
    import numpy as _np
    from concourse import dve_ops as DO
    from concourse.dve_spec import Spec, Src0, C0, C1, C2, One, sq, lower
    from concourse.dve_ops import has_src1
    from concourse.dve_uop import DveOpSpec

    if "EXP32_POLY_ANT" in DO._SUB_OPCODE_FOR_NAME:
        by = {op.name: op for op in DO.OPS}
        return by["EXP32_POLY_ANT"], by["EXP32_SQ_ANT"]

    s = Src0
    specs = [
        ("EXP32_POLY_ANT", Spec(
            body=((s * C2 + C1) * s + C0) * s + One,
            reference=lambda in0, in1, s0, s1, imm2: (
                (in0 * imm2 + s1) * in0 + s0) * in0 + 1.0)),
        ("EXP32_SQ_ANT", Spec(
            body=sq(sq(sq(sq(sq(s))))),
            reference=lambda in0, in1, s0, s1, imm2: (
                in0.astype(_np.float64) ** 32))),
    ]
    ops = []
    for name, spec in specs:
        op = DO.DveOp(name, spec, subdim=False, uops_sha={})
        DO.OPS.append(op)
        DO._SUB_OPCODE_FOR_NAME[name] = DO._CUSTOM_DVE_ROW_BASE + len(DO.OPS) - 1
        DO.CUSTOM_DVE_SPECS[name] = spec
        so = DveOpSpec(name=name, opcode=DO.get_dve_sub_opcode(name),
                       uops=lower(spec, ver="v3"), rd1_en=has_src1(spec))
        op.uops_sha["v3"] = so.sha("v3")
        ops.append(op)
    assert max(DO._SUB_OPCODE_FOR_NAME.values()) < 0x20
    return ops[0], ops[1]


def _build_program():
    """Build (and cache) the SPMD Bass program. Returns (nc, names)."""
    from contextlib import ExitStack

    import concourse.bass as bass
    import concourse.mybir as mybir
    import concourse.tile as tile
    from concourse import bacc

    f32 = mybir.dt.float32
    bf16 = mybir.dt.bfloat16
    AF = mybir.ActivationFunctionType
    OP = mybir.AluOpType

    xp_op, xs_op = _register_dve_exp()

    nc = bacc.Bacc(None, target_bir_lowering=False)

    # ---- DRAM I/O ----------------------------------------------------
    x_all = nc.dram_tensor("x_all", [P, NKT, D], f32, kind="ExternalInput")
    xqbp = nc.dram_tensor("xqbp", [P, NQT, D], f32, kind="ExternalInput")
    wq_d = nc.dram_tensor("wq", [P, C, H * E], bf16, kind="ExternalInput")
    wk_d = nc.dram_tensor("wk", [P, C, H * E], bf16, kind="ExternalInput")
    wv_d = nc.dram_tensor("wv", [P, C, H * E], bf16, kind="ExternalInput")
    wp_d = nc.dram_tensor("wp", [P, C, D], bf16, kind="ExternalInput")
    w1_d = nc.dram_tensor("w1", [P, C, F], bf16, kind="ExternalInput")
    w2_d = nc.dram_tensor("w2", [P, FC, D], bf16, kind="ExternalInput")
    bq_d = nc.dram_tensor("bq_c", [P, C], f32, kind="ExternalInput")
    bk_d = nc.dram_tensor("bk_c", [P, C], f32, kind="ExternalInput")
    bv_d = nc.dram_tensor("bv_b", [P, H * E], f32, kind="ExternalInput")
    b1_d = nc.dram_tensor("b1_c", [P, FC], f32, kind="ExternalInput")
    b2_d = nc.dram_tensor("b2_b", [P, D], f32, kind="ExternalInput")
    id_d = nc.dram_tensor("ident", [P, P], bf16, kind="ExternalInput")
    f32r = mybir.dt.float32r
    on_d = nc.dram_tensor("ones64", [1, E], f32r, kind="ExternalInput")
    y_out = nc.dram_tensor("y_out", [P, NQT, D], f32, kind="ExternalOutput")

    with tile.TileContext(nc) as tc, ExitStack() as ctx:
        pers = ctx.enter_context(tc.tile_pool(name="pers", bufs=1))
        px = ctx.enter_context(tc.tile_pool(name="px", bufs=CFG["px_bufs"]))
        pxn = ctx.enter_context(tc.tile_pool(name="pxn", bufs=CFG["pxn_bufs"]))
        pexp = ctx.enter_context(tc.tile_pool(name="pexp", bufs=CFG["pexp_bufs"]))
        ptmp = ctx.enter_context(tc.tile_pool(name="ptmp", bufs=CFG["ptmp_bufs"]))
        pst = ctx.enter_context(tc.tile_pool(name="pst", bufs=8))
        prr = ctx.enter_context(tc.tile_pool(name="prr", bufs=1))
        ps_big = ctx.enter_context(tc.tile_pool(name="ps_big", bufs=CFG["ps_big_bufs"], space="PSUM"))
        ps_att = ctx.enter_context(tc.tile_pool(name="ps_att", bufs=CFG["ps_att_bufs"], space="PSUM"))

        # ---- persistent SBUF tensors --------------------------------
        def pt(shape, dt, tag):
            return pers.tile(shape, dt, tag=tag, name=tag)

        w_q = pt([P, C, H * E], bf16, "w_q")
        w_k = pt([P, C, H * E], bf16, "w_k")
        w_v = pt([P, C, H * E], bf16, "w_v")
        w_p = pt([P, C, D], bf16, "w_p")
        w_1 = pt([P, C, F], bf16, "w_1")
        w_2 = pt([P, FC, D], bf16, "w_2")
        bq_c = pt([P, C], f32, "bq_c")
        bk_c = pt([P, C], f32, "bk_c")
        bv_b = pt([P, H * E], f32, "bv_b")
        b1_c = pt([P, FC], f32, "b1_c")
        b2_b = pt([P, D], f32, "b2_b")
        ident = pt([P, P], bf16, "ident")
        ones64 = pt([1, E], f32r, "ones64")
        xnT = pt([P, C, S], bf16, "xnT")
        qT = pt([P, C, SQ], bf16, "qT")
        kT = pt([P, C, S], bf16, "kT")
        v_sb = pt([P, NKT, H * EA], bf16, "v_sb")
        attnT = pt([P, C, SQ], bf16, "attnT")
        x1_sb = pt([P, NQT, D], f32, "x1_sb")
        x1nT = pt([P, C, SQ], bf16, "x1nT")
        hT = pt([P, FC, SQ], bf16, "hT")

        for dst, src in [
            (ident, id_d), (w_q, wq_d), (w_k, wk_d), (w_v, wv_d),
            (bq_c, bq_d), (bk_c, bk_d), (bv_b, bv_d), (ones64, on_d),
            (w_p, wp_d), (b1_c, b1_d), (b2_b, b2_d),
            (w_1, w1_d), (w_2, w2_d),
        ]:
            nc.sync.dma_start(dst[:], src[:])

        # ---- helper: layernorm stats -> (mean, rstd) ----------------
        def norm_stats(xt):
            st6 = pst.tile([P, 6], f32, tag="st6", name="st6")
            nc.vector.bn_stats(st6[:], xt)
            mv = pst.tile([P, 2], f32, tag="mv", name="mv")
            nc.vector.bn_aggr(mv[:], st6[:])
            std = pst.tile([P, 1], f32, tag="std", name="std")
            nc.scalar.activation(std[:], mv[:, 1:2], AF.Sqrt, scale=BESSEL)
            rstd = pst.tile([P, 1], f32, tag="rstd", name="rstd")
            nc.vector.reciprocal(rstd[:], std[:])
            return mv, rstd

        # transpose a [P, D] bf16 tile into dstT[:, :, tcol*P : +P]
        def transpose_into(dstT, xn, tcol):
            if CFG["tr_pool"] == "att":
                ps = ps_att.tile([P, 512], bf16, tag="att", name="tr")
            else:
                ps = ps_big.tile([P, 512], bf16, tag="mm", name="tr")
            for c in range(C):
                nc.tensor.transpose(
                    ps[:, c * P:(c + 1) * P], xn[:, c * P:(c + 1) * P], ident[:]
                )
            nc.scalar.copy(
                dstT[:, :, tcol * P:(tcol + 1) * P],
                ps[:].rearrange("p (c j) -> p c j", c=C),
            )

        # ---- phase A: norm1 + transpose ------------------------------
        for t in range(NKT):
            xt = px.tile([P, D], f32, tag="x", name="x")
            nc.gpsimd.dma_start(xt[:], x_all[:, t, :])
            mv, rstd = norm_stats(xt[:])
            xn = pxn.tile([P, D], bf16, tag="xn", name="xn")
            nc.gpsimd.tensor_scalar(
                xn[:], xt[:], mv[:, 0:1], rstd[:], OP.subtract, OP.mult
            )
            transpose_into(xnT, xn[:], t)
            # V for tile t needs only this tile's xnT columns -> emit now
            ps = ps_att.tile([P, 512], f32, tag="att", name="vps")
            for ci in range(C):
                nc.tensor.matmul(
                    ps[:],
                    xnT[:, ci, t * P:(t + 1) * P],
                    w_v[:, ci, :],
                    start=(ci == 0), stop=(ci == C - 1),
                )
            vt = v_sb[:, t, :].rearrange("p (h e) -> p h e", h=H)
            nc.vector.tensor_tensor(
                vt[:, :, 0:E],
                ps[:].rearrange("p (h e) -> p h e", h=H),
                bv_b[:].rearrange("p (h e) -> p h e", h=H),
                OP.add,
            )
            nc.vector.memset(vt[:, :, E:EA], 1.0)

        # ---- phase B: QKV projections -------------------------------
        # Q^T / K^T: [HE, tokens] = sum_c W[:,c,:].T @ xnT[:,c,:]
        def proj_qk(w, dstT, bias_c, co, n0, ntiles):
            # [128, 1024] psum = two 512-wide accumulation groups; one
            # DVE eviction (psum + per-partition bias -> bf16)
            ps = ps_big.tile([P, 1024], f32, tag="mm", name="mm")
            for half in range(2):
                for ci in range(C):
                    nc.tensor.matmul(
                        ps[:, half * 512:(half + 1) * 512],
                        w[:, ci, co * P:(co + 1) * P],
                        xnT[:, ci, (n0 + half) * 512:(n0 + half + 1) * 512],
                        start=(ci == 0), stop=(ci == C - 1),
                    )
            nc.vector.tensor_scalar(
                dstT[:, co, n0 * 512:(n0 + 2) * 512], ps[:],
                bias_c[:, co:co + 1], None, OP.add,
            )

        # n-outer: the n=0 projections only need token tiles 0-3
        proj_qk(w_q, qT, bq_c, 0, 0, 2)
        proj_qk(w_k, kT, bk_c, 0, 0, 2)
        for co in range(1, C):
            proj_qk(w_q, qT, bq_c, co, 0, 2)
            proj_qk(w_k, kT, bk_c, co, 0, 2)
        for co in range(C):
            proj_qk(w_k, kT, bk_c, co, 2, 2)

        # ---- phase C: attention per head ----------------------------
        # The head-boundary normalize is split: the DVE part (recip +
        # copy) runs right after the last att GEMM, but the broadcast
        # matmul + final multiply are deferred into the NEXT head's kt
        # loop, so the PE never stalls in program order waiting on DVE.
        def finish_head(h, att_un, rr):
            ch, off = h // 2, (h % 2) * E
            bc = ps_big.tile([E, SQ], f32, tag="mm", name="mm")
            for n in range(SQ // 512):
                nc.tensor.matmul(
                    bc[:, n * 512:(n + 1) * 512], ones64[:],
                    rr[:, n * 512:(n + 1) * 512],
                    start=True, stop=True,
                )
            nc.vector.tensor_tensor(
                attnT[off:off + E, ch, :], att_un[:], bc[:], OP.mult
            )

        deferred = None
        for h in range(H):
            ch, off = h // 2, (h % 2) * E
            att = ps_att.tile([EA, SQ], f32, tag="att", name="att")

            def att_mm(kt, ex):
                for n in range(SQ // 512):
                    nc.tensor.matmul(
                        att[:, n * 512:(n + 1) * 512],
                        v_sb[:, kt, h * EA:(h + 1) * EA],
                        ex[:, n * 512:(n + 1) * 512],
                        start=(kt == 0), stop=(kt == NKT - 1),
                    )

            pending = None
            for kt in range(NKT):
                if CFG["scs_alt"] and kt % 2 == 1:
                    scs = ps_att.tile([P, SQ], f32, tag="att", name="scs")
                else:
                    scs = ps_big.tile([P, SQ], f32, tag="mm", name="mm")
                for n in range(SQ // 512):
                    nc.tensor.matmul(
                        scs[:, n * 512:(n + 1) * 512],
                        kT[off:off + E, ch, kt * P:(kt + 1) * P],
                        qT[off:off + E, ch, n * 512:(n + 1) * 512],
                        start=True, stop=True,
                    )
                ex = pexp.tile([P, SQ], bf16, tag="ex", name="ex")
                m = CFG["dve_exp_mod"]
                if m and kt % m == m - 1:
                    nc.vector._custom_dve(
                        xp_op, out=scs[:], in0=scs[:], s0=EC1, s1=EC2, imm2=EC3
                    )
                    nc.vector._custom_dve(xs_op, out=ex[:], in0=scs[:])
                else:
                    nc.scalar.activation(
                        ex[:], scs[:], AF.Exp, scale=float(SCALE)
                    )
                if pending is not None:
                    att_mm(kt - 1, pending)
                pending = ex
                if kt == 2 and deferred is not None:
                    finish_head(*deferred)
                    deferred = None
            att_mm(NKT - 1, pending)
            # immediate DVE part: recip first (bcast only needs this),
            # then the att_un eviction copy
            rrt = prr.tile([1, SQ], f32r, tag="rr", name="rr")
            with nc.allow_low_precision(
                reason="softmax denom recip rounded to f32r for the "
                "broadcast matmul; ~1e-6 relative"
            ):
                nc.vector.reciprocal(rrt[:], att[E:EA, :])
            att_un = ptmp.tile([E, SQ], f32, tag="tmp", name="tmp")
            nc.vector.tensor_copy(att_un[:], att[0:E, :])
            deferred = (h, att_un, rrt[:])
        finish_head(*deferred)

        # ---- phase D: projection + residual + norm2 -----------------
        # pipelined one qt deep: the PE transposes of qt wait on a DVE
        # stats chain, so qt+1's projection matmuls are emitted first
        d_pend = None
        for qt in range(NQT):
            ps = ps_big.tile([P, 512], f32, tag="mm", name="mm")
            for c in range(C):
                nc.tensor.matmul(
                    ps[:],
                    attnT[:, c, qt * P:(qt + 1) * P],
                    w_p[:, c, :],
                    start=(c == 0), stop=(c == C - 1),
                )
            xq = px.tile([P, D], f32, tag="x", name="x")
            nc.sync.dma_start(xq[:], xqbp[:, qt, :])
            nc.vector.tensor_tensor(x1_sb[:, qt, :], ps[:], xq[:], OP.add)
            mv, rstd = norm_stats(x1_sb[:, qt, :])
            x1n = pxn.tile([P, D], bf16, tag="xn", name="xn")
            nc.gpsimd.tensor_scalar(
                x1n[:], x1_sb[:, qt, :], mv[:, 0:1], rstd[:], OP.subtract, OP.mult
            )
            if d_pend is not None:
                transpose_into(x1nT, d_pend[1], d_pend[0])
            d_pend = (qt, x1n[:])
        transpose_into(x1nT, d_pend[1], d_pend[0])

        # ---- phase E: FFN1 + gelu -----------------------------------
        for n in range(SQ // 512):
            for fc in range(FC):
                ps = ps_big.tile([P, 512], f32, tag="mm", name="mm")
                for c in range(C):
                    nc.tensor.matmul(
                        ps[:],
                        w_1[:, c, fc * P:(fc + 1) * P],
                        x1nT[:, c, n * 512:(n + 1) * 512],
                        start=(c == 0), stop=(c == C - 1),
                    )
                nc.scalar.activation(
                    hT[:, fc, n * 512:(n + 1) * 512], ps[:],
                    AF.Gelu, bias=b1_c[:, fc:fc + 1],
                )

        # ---- phase F: FFN2 + gelu + residual ------------------------
        for qt in range(NQT):
            ps = ps_big.tile([P, 512], f32, tag="mm", name="mm")
            for fc in range(FC):
                nc.tensor.matmul(
                    ps[:],
                    hT[:, fc, qt * P:(qt + 1) * P],
                    w_2[:, fc, :],
                    start=(fc == 0), stop=(fc == FC - 1),
                )
            pre2 = ptmp.tile([P, D], f32, tag="tmp", name="tmp")
            nc.vector.tensor_tensor(pre2[:], ps[:], b2_b[:], OP.add)
            g2 = ptmp.tile([P, D], f32, tag="tmp", name="tmp")
            nc.scalar.activation(g2[:], pre2[:], AF.Gelu)
            yt = ptmp.tile([P, D], f32, tag="tmp", name="tmp")
            nc.vector.tensor_tensor(yt[:], g2[:], x1_sb[:, qt, :], OP.add)
            nc.sync.dma_start(y_out[:, qt, :], yt[:])

    nc.compile()
    return nc


def _pack_pmajor(a, ntiles):
    """[ntiles*128, W] -> [128, ntiles, W] with tile t, partition p = row t*128+p."""
    return np.ascontiguousarray(a.reshape(ntiles, P, -1).transpose(1, 0, 2))


def _prep_shared(Wq, bq, Wk, bk, Wv, bv, Wp, gamma1, beta1, gamma2, beta2,
                 W1, b1, W2, b2):
    g1 = np.asarray(gamma1, np.float64)
    be1 = np.asarray(beta1, np.float64)
    g2 = np.asarray(gamma2, np.float64)
    be2 = np.asarray(beta2, np.float64)

    def headcat(w):  # [H, D, E] -> [D, H*E]
        return np.ascontiguousarray(
            np.transpose(np.asarray(w, np.float64), (1, 0, 2)).reshape(D, H * E)
        )

    out = {}
    for name, w, b in [("q", Wq, bq), ("k", Wk, bk)]:
        wa = headcat(w)
        beff = np.asarray(b, np.float64).reshape(-1) + be1 @ wa
        wag = wa * g1[:, None]
        out["w" + name] = _pack_pmajor(wag, C).astype(BF16)
        out["b" + name + "_c"] = np.ascontiguousarray(
            beff.reshape(C, P).T
        ).astype(np.float32)
    wv_a = headcat(Wv)
    bv_eff = np.asarray(bv, np.float64).reshape(-1) + be1 @ wv_a
    out["wv"] = _pack_pmajor(wv_a * g1[:, None], C).astype(BF16)
    out["bv_b"] = np.ascontiguousarray(
        np.broadcast_to(bv_eff.astype(np.float32), (P, H * E))
    )
    out["wp"] = _pack_pmajor(np.asarray(Wp, np.float64), C).astype(BF16)
    w1_a = np.asarray(W1, np.float64)
    b1_eff = np.asarray(b1, np.float64) + be2 @ w1_a
    out["w1"] = _pack_pmajor(w1_a * g2[:, None], C).astype(BF16)
    out["b1_c"] = np.ascontiguousarray(b1_eff.reshape(FC, P).T).astype(np.float32)
    out["w2"] = _pack_pmajor(np.asarray(W2, np.float64), FC).astype(BF16)
    out["b2_b"] = np.ascontiguousarray(
        np.broadcast_to(np.asarray(b2, np.float32), (P, D))
    )
    out["ident"] = np.eye(P, dtype=BF16)
    out["ones64"] = np.ones((1, E), dtype=np.float32)
    return out


def _gather(results):
    y = np.empty((B, S, D), np.float32)
    for core in range(8):
        b_idx, half = core // 2, core % 2
        yp = np.asarray(results[core]["y_out"], np.float32)
        y[b_idx, half * SQ:(half + 1) * SQ] = (
            yp.transpose(1, 0, 2).reshape(SQ, D)
        )
    return y.reshape(B, S, D, 1, 1)


def kernel(x, Wq, bq, Wk, bk, Wv, bv, Wp, bp, gamma1, beta1, gamma2, beta2,
           W1, b1, W2, b2):
    from concourse.bass_utils import run_bass_kernel_spmd

    if "nc" not in _CACHE:
        _CACHE["nc"] = _build_program()
    nc = _CACHE["nc"]

    weights = dict(
        Wq=Wq, bq=bq, Wk=Wk, bk=bk, Wv=Wv, bv=bv, Wp=Wp,
        gamma1=gamma1, beta1=beta1, gamma2=gamma2, beta2=beta2,
        W1=W1, b1=b1, W2=W2, b2=b2,
    )
    x_flat = np.asarray(x, np.float32).reshape(B, S, D)
    shared = _prep_shared(**weights)
    bp_a = np.asarray(bp, np.float32)
    in_maps = []
    for core in range(8):
        b_idx, half = core // 2, core % 2
        xo = np.roll(x_flat[b_idx], -half * SQ, axis=0)
        m = dict(shared)
        m["x_all"] = _pack_pmajor(xo, NKT)
        m["xqbp"] = _pack_pmajor(xo[:SQ] + bp_a[None, :], NQT)
        in_maps.append(m)

    res = run_bass_kernel_spmd(nc, in_maps, core_ids=list(range(8)))
    return _gather(res.results)
